# revision 16
# baseline (speedup 1.0000x reference)
"""Trainium2 Bass kernel for Cascade R-CNN box head (ROIAlign over FPN + 3-layer MLP).

Sharding: data-parallel over ROIs across 8 NeuronCores, stratified by pixel-table
window so all cores run one SPMD program; FC weights replicated (fp16).

Per core the device:
  1. dma_gather's each ROI's unique bilinear-corner pixel vectors [pix, C=256] (fp16)
     from an NHWC pixel table.
  2. Pools via PE matmuls: lhsT = pixel tile (stationary), rhs = host-built separable
     ROIAlign weights [pix, 49] -> PSUM [c_half, 49], accumulated over the ROI's tiles.
  3. Assembles pooled^T in [c', (roi, k)] layout and runs fp16 GEMMs:
     12544->1024 +relu, 1024->1024 +relu, 1024->20, with PE transposes between.
"""
import numpy as np

OUT = 7
RATIO = 2
P = OUT * RATIO  # 14 sample points per dim
STRIDES = (4, 8, 16, 32)
SHAPES = {0: (192, 320), 1: (96, 160), 2: (48, 80), 3: (24, 40)}
B = 2
C = 256
NBINS = OUT * OUT  # 49
N_CORES = 8
MAX_CHUNK_TILES = 24  # pixel tiles per dma_gather chunk (24*128*512B = 1.5MB)

_BUILD_CACHE = {}
LAST_RESULT = None

# ---------------------------------------------------------------- windows
_LVL_BASE = {}
_base = 0
for _l in range(4):
    _H, _W = SHAPES[_l]
    _LVL_BASE[_l] = _base
    _base += B * _H * _W
TOTAL_ROWS = _base  # 163200 (Y-major: row = lvl_base + b*H*W + y*W + x)
# L0 is additionally stored X-major (row = X_BASE + x*(B*H) + b*H + y) for
# tall-skinny ROIs: at lvl0 span_x*span_y < 784 so at most one dim is long.
X_BASE = TOTAL_ROWS
TOTAL_ROWS_EXT = TOTAL_ROWS + B * 192 * 320  # 286080

# (start, nrows). Y-major L0 windows: ROI fits iff its row span <= 31 (idx span
# <= 30*320+319 = 9919); stride 32768-9920. X-major: x-extent <= 30 -> idx span
# <= 29*384+191 < 11328; stride 32768-11520.
WINDOWS = []
_l0_end = _LVL_BASE[1]
_s = 0
while True:
    if _s + 32768 >= _l0_end:
        WINDOWS.append((_l0_end - 32768, 32768))
        break
    WINDOWS.append((_s, 32768))
    _s += 32768 - 9920
_N_YWIN = len(WINDOWS)
_s = 0
while True:
    if _s + 32768 >= B * 192 * 320:
        WINDOWS.append((X_BASE + B * 192 * 320 - 32768, 32768))
        break
    WINDOWS.append((X_BASE + _s, 32768))
    _s += 32768 - 11520
WINDOWS.append((_LVL_BASE[1], B * 96 * 160))          # L1: 30720 rows
WINDOWS.append((_LVL_BASE[2], B * (48 * 80 + 24 * 40)))  # L2+L3: 9600 rows


def _find_window(lo, hi):
    for wi, (start, nrows) in enumerate(WINDOWS):
        if lo >= start and hi < start + nrows:
            return wi
    return -1


# ---------------------------------------------------------------- host-side math
def _axis_r(coord, size):
    """Mirror of reference._bilinear_axis, collapsed to unique indices.
    Returns unique indices U and R [len(U), 7] with the 0.5-bin-average folded in."""
    coord = coord.astype(np.float32)
    valid = ((coord > -1.0) & (coord < size)).astype(np.float32)
    c = np.clip(coord, np.float32(0.0), np.float32(size - 1))
    i0 = np.floor(c).astype(np.int32)
    i1 = np.minimum(i0 + 1, size - 1)
    frac = (c - i0.astype(np.float32)).astype(np.float32)
    w0 = ((np.float32(1.0) - frac) * valid).astype(np.float32)
    w1 = (frac * valid).astype(np.float32)
    uniq = np.unique(np.concatenate([i0, i1]))
    pos = {int(v): k for k, v in enumerate(uniq)}
    R = np.zeros((len(uniq), OUT), dtype=np.float32)
    for s in range(P):
        o = s // RATIO
        R[pos[int(i0[s])], o] += np.float32(0.5) * w0[s]
        R[pos[int(i1[s])], o] += np.float32(0.5) * w1[s]
    return uniq, R


def _prep_rois(rois):
    rois = np.asarray(rois, dtype=np.float32)
    n = rois.shape[0]
    bidx = rois[:, 0].astype(np.int32)
    boxes = rois[:, 1:5].astype(np.float32)
    w = boxes[:, 2] - boxes[:, 0]
    h = boxes[:, 3] - boxes[:, 1]
    k = np.floor(np.float32(4.0) + np.log2(np.sqrt(np.maximum(w * h, np.float32(1e-6))) / np.float32(224.0)))
    lvl = np.clip(k, 2.0, 5.0).astype(np.int32) - 2
    steps = ((np.arange(P, dtype=np.float32) + np.float32(0.5)) / np.float32(P)).astype(np.float32)
    per_roi = []
    for i in range(n):
        l = int(lvl[i])
        stride = np.float32(STRIDES[l])
        H, W = SHAPES[l]
        x1 = boxes[i, 0] / stride - np.float32(0.5)
        y1 = boxes[i, 1] / stride - np.float32(0.5)
        x2 = boxes[i, 2] / stride - np.float32(0.5)
        y2 = boxes[i, 3] / stride - np.float32(0.5)
        px = (x1 + (x2 - x1) * steps).astype(np.float32)
        py = (y1 + (y2 - y1) * steps).astype(np.float32)
        ux, RX = _axis_r(px, W)
        uy, RY = _axis_r(py, H)
        base = _LVL_BASE[l] + int(bidx[i]) * H * W
        gidx = (base + uy[:, None].astype(np.int64) * W + ux[None, :]).ravel()
        wi = _find_window(int(gidx.min()), int(gidx.max()))
        if wi >= 0:
            Wd = np.einsum("yo,xp->yxop", RY, RX).reshape(-1, NBINS)
        else:
            # X-major fallback (lvl0 tall-skinny): row = X_BASE + x*384 + b*192 + y
            assert l == 0, (l, int(gidx.min()), int(gidx.max()))
            gidx = (X_BASE + ux[:, None].astype(np.int64) * (B * H)
                    + int(bidx[i]) * H + uy[None, :]).ravel()
            wi = _find_window(int(gidx.min()), int(gidx.max()))
            assert wi >= 0, (int(gidx.min()), int(gidx.max()))
            Wd = np.einsum("xp,yo->xyop", RX, RY).reshape(-1, NBINS)
        npix = len(gidx)
        per_roi.append(dict(rid=i, win=wi, npix=npix, ntiles=(npix + 127) // 128,
                            lidx=(gidx - WINDOWS[wi][0]).astype(np.int16), Wd=Wd))
    return per_roi


def _pack(per_roi):
    """structure: list of (window, [per-slot tile counts]) shared by all cores;
    core_slots: per core, per global slot, the roi dict or None (dummy)."""
    by_win = {}
    for r in per_roi:
        by_win.setdefault(r["win"], []).append(r)
    structure = []
    core_slots = [[] for _ in range(N_CORES)]
    for wi in sorted(by_win):
        rs = sorted(by_win[wi], key=lambda r: -r["ntiles"])
        nslots = (len(rs) + N_CORES - 1) // N_CORES
        tile_counts = []
        for s in range(nslots):
            group = rs[s * N_CORES:(s + 1) * N_CORES]
            tile_counts.append(group[0]["ntiles"])
            for c in range(N_CORES):
                core_slots[c].append(group[c] if c < len(group) else None)
        structure.append((wi, tile_counts))
    return structure, core_slots


def _chunk_plan(tile_counts):
    """Greedy chunks of whole slots, each <= MAX_CHUNK_TILES tiles.
    Returns list of (chunk_tile_count, idx_col_offset, idx_cols, first_slot, n_slots)."""
    plan = []
    cur_slots = 0
    cur_t = 0
    first = 0
    col = 0
    for k, t in enumerate(tile_counts):
        assert t <= MAX_CHUNK_TILES
        if cur_slots and cur_t + t > MAX_CHUNK_TILES:
            plan.append((cur_t, col, cur_t * 8, first, cur_slots))
            col += cur_t * 8
            first, cur_slots, cur_t = k, 0, 0
        cur_slots += 1
        cur_t += t
    if cur_slots:
        plan.append((cur_t, col, cur_t * 8, first, cur_slots))
    return plan


def _build_core_data(structure, slots):
    idx_arrays = []
    w_blocks = []
    roi_ids = []
    si = 0
    for wi, tile_counts in structure:
        plan = _chunk_plan(tile_counts)
        win_cols = []
        for (ct, col_off, cols, first, ns) in plan:
            nidx = ct * 128
            ids = np.zeros(nidx, dtype=np.int16)
            off = 0
            for k in range(first, first + ns):
                s = slots[si + k]
                if s is not None:
                    ids[off:off + s["npix"]] = s["lidx"]
                off += tile_counts[k] * 128
            wrapped = ids.reshape(nidx // 16, 16).T.copy()  # idx i -> [i%16, i//16]
            win_cols.append(wrapped)
        wrapped_all = (np.concatenate(win_cols, axis=1) if win_cols
                       else np.zeros((16, 0), np.int16))
        idx_arrays.append(np.ascontiguousarray(np.tile(wrapped_all, (8, 1))))
        for k, t in enumerate(tile_counts):
            s = slots[si + k]
            blk = np.zeros((t * 128, NBINS), dtype=np.float16)
            if s is not None:
                blk[:s["npix"]] = s["Wd"].astype(np.float16)
            w_blocks.append(blk)
            roi_ids.append(s["rid"] if s is not None else -1)
        si += len(tile_counts)
    W_all = np.concatenate(w_blocks, axis=0)              # [T*128, 49]
    T = W_all.shape[0] // 128
    W_sw = np.ascontiguousarray(
        W_all.reshape(T, 128, NBINS).transpose(1, 0, 2).reshape(128, T * NBINS)
    ).astype(np.float16)
    return idx_arrays, W_sw, roi_ids


# ---------------------------------------------------------------- device program
def _build_program(structure, idx_shapes, n_wtiles, n_batches):
    import concourse.bacc as bacc
    import concourse.mybir as mybir
    import concourse.tile as tile
    from concourse._compat import get_trn_type
    from concourse.masks import make_identity

    fp16 = mybir.dt.float16
    fp32 = mybir.dt.float32
    RELU = mybir.ActivationFunctionType.Relu

    nc = bacc.Bacc(get_trn_type() or "TRN2", num_swdge_queues=4)
    table = nc.dram_tensor("table", [TOTAL_ROWS_EXT, C], fp16, kind="ExternalInput")
    idx_ts = [nc.dram_tensor(f"idx{w}", [128, max(sh, 1)], mybir.dt.int16, kind="ExternalInput")
              for w, sh in enumerate(idx_shapes)]
    w_pool = nc.dram_tensor("w_pool", [128, n_wtiles * NBINS], fp16, kind="ExternalInput")
    w1 = nc.dram_tensor("w1", [98 * 128, 1024], fp16, kind="ExternalInput")
    w2 = nc.dram_tensor("w2", [8 * 128, 1024], fp16, kind="ExternalInput")
    w3 = nc.dram_tensor("w3", [128, 8 * 32], fp16, kind="ExternalInput")  # swizzled
    # biases: [fc1_b(1024) fc2_b(1024) p_b(32) ones(128)]
    biases = nc.dram_tensor("biases", [1, 2 * 1024 + 32 + 128], fp16, kind="ExternalInput")
    out_t = nc.dram_tensor("out", [256, 20], fp32, kind="ExternalOutput")

    with tile.TileContext(nc) as tc:
        with (
            tc.tile_pool(name="persist", bufs=1) as persist,
            tc.tile_pool(name="pix", bufs=3) as pixp,
            tc.tile_pool(name="wts", bufs=3) as wtsp,
            tc.tile_pool(name="idxp", bufs=2) as idxp,
            tc.tile_pool(name="pool_ps", bufs=4, space="PSUM") as psp,
            tc.tile_pool(name="mm_ps", bufs=4, space="PSUM") as psmm,
            tc.tile_pool(name="stream", bufs=16) as streamp,
            tc.tile_pool(name="small", bufs=1) as smallp,
        ):
            pooledT = [persist.tile([128, 128, 98], fp16, tag=f"pooledT{b}", name=f"pooledT{b}")
                       for b in range(n_batches)]
            for b in range(n_batches):
                eng = nc.vector if b == 0 else nc.gpsimd
                eng.memset(pooledT[b][:], 0.0)
            ident = persist.tile([128, 128], fp16, tag="ident")
            make_identity(nc, ident[:])
            bias_sb = persist.tile([1, 2 * 1024 + 32 + 128], fp16, tag="bias")
            nc.sync.dma_start(out=bias_sb[:], in_=biases[:])
            ones = bias_sb[:, 2080:2208]  # [1, 128] of 1.0

            # ---------------- pooling ----------------
            slot_base = 0
            wtile_base = 0
            gq = 0
            for we, (wi, tile_counts) in enumerate(structure):
                win_start, win_rows = WINDOWS[wi]
                win_ap = table[win_start:win_start + win_rows, :]
                idx_sb = idxp.tile([128, max(idx_shapes[we], 1)], mybir.dt.int16, tag="idx")
                nc.sync.dma_start(out=idx_sb[:], in_=idx_ts[we][:])
                for (ct, col_off, cols, first, ns) in _chunk_plan(tile_counts):
                    pix = pixp.tile([128, MAX_CHUNK_TILES, C], fp16, tag="pix")
                    nc.gpsimd.dma_gather(
                        pix[:, :ct, :], win_ap, idx_sb[:, col_off:col_off + cols],
                        ct * 128, ct * 128, C, single_packet=False,
                        queue_num=gq % 4,
                    )
                    gq += 1
                    wts = wtsp.tile([128, MAX_CHUNK_TILES * NBINS], fp16, tag="wts")
                    nc.sync.dma_start(
                        out=wts[:, :ct * NBINS],
                        in_=w_pool[:, wtile_base * NBINS:(wtile_base + ct) * NBINS],
                    )
                    t_off = 0
                    for k in range(first, first + ns):
                        t = tile_counts[k]
                        ps0 = psp.tile([128, NBINS], fp32, tag="pool_ps")
                        ps1 = psp.tile([128, NBINS], fp32, tag="pool_ps")
                        for tt in range(t):
                            rhs = wts[:, (t_off + tt) * NBINS:(t_off + tt + 1) * NBINS]
                            nc.tensor.matmul(out=ps0[:], lhsT=pix[:, t_off + tt, 0:128],
                                             rhs=rhs, start=(tt == 0), stop=(tt == t - 1))
                            nc.tensor.matmul(out=ps1[:], lhsT=pix[:, t_off + tt, 128:256],
                                             rhs=rhs, start=(tt == 0), stop=(tt == t - 1))
                        r = slot_base + k
                        dst = pooledT[r // 128]
                        nc.vector.tensor_copy(out=dst[:, r % 128, 0:49], in_=ps0[:])
                        nc.scalar.copy(out=dst[:, r % 128, 49:98], in_=ps1[:])
                        t_off += t
                    wtile_base += ct
                slot_base += len(tile_counts)

            # ---------------- fc1 ----------------
            fc1_ps = [[psmm.tile([128, 512], fp32, tag="mm", name=f"fc1ps{b}_{h}") for h in range(2)]
                      for b in range(n_batches)]
            for kk in range(98):
                w1_sb = streamp.tile([128, 1024], fp16, tag="wstream")
                nc.sync.dma_start(out=w1_sb[:], in_=w1[kk * 128:(kk + 1) * 128, :])
                for b in range(n_batches):
                    nc.tensor.matmul(out=fc1_ps[b][0][:], lhsT=pooledT[b][:, :, kk],
                                     rhs=w1_sb[:, 0:512], start=(kk == 0), stop=False)
                    nc.tensor.matmul(out=fc1_ps[b][1][:], lhsT=pooledT[b][:, :, kk],
                                     rhs=w1_sb[:, 512:1024], start=(kk == 0), stop=False)
            fc1r = [smallp.tile([128, 1024], fp16, tag=f"fc1r{b}", name=f"fc1r{b}") for b in range(n_batches)]
            for b in range(n_batches):
                nc.tensor.matmul(out=fc1_ps[b][0][:], lhsT=ones, rhs=bias_sb[:, 0:512],
                                 start=False, stop=True)
                nc.tensor.matmul(out=fc1_ps[b][1][:], lhsT=ones, rhs=bias_sb[:, 512:1024],
                                 start=False, stop=True)
                nc.scalar.activation(out=fc1r[b][:, 0:512], in_=fc1_ps[b][0][:], func=RELU)
                nc.scalar.activation(out=fc1r[b][:, 512:1024], in_=fc1_ps[b][1][:], func=RELU)

            fc1rT = [smallp.tile([128, 8, 128], fp16, tag=f"fc1rT{b}", name=f"fc1rT{b}") for b in range(n_batches)]
            for b in range(n_batches):
                for kk in range(8):
                    tp = psmm.tile([128, 128], fp16, tag="mm", name=f"tp1_{b}_{kk}")
                    nc.tensor.transpose(out=tp[:], in_=fc1r[b][:, kk * 128:(kk + 1) * 128],
                                        identity=ident[:])
                    if kk % 2 == 0:
                        nc.vector.tensor_copy(out=fc1rT[b][:, kk, :], in_=tp[:])
                    else:
                        nc.scalar.copy(out=fc1rT[b][:, kk, :], in_=tp[:])

            # ---------------- fc2 ----------------
            fc2_ps = [[psmm.tile([128, 512], fp32, tag="mm", name=f"fc2ps{b}_{h}") for h in range(2)]
                      for b in range(n_batches)]
            for kk in range(8):
                w2_sb = streamp.tile([128, 1024], fp16, tag="wstream")
                nc.sync.dma_start(out=w2_sb[:], in_=w2[kk * 128:(kk + 1) * 128, :])
                for b in range(n_batches):
                    nc.tensor.matmul(out=fc2_ps[b][0][:], lhsT=fc1rT[b][:, kk, :],
                                     rhs=w2_sb[:, 0:512], start=(kk == 0), stop=False)
                    nc.tensor.matmul(out=fc2_ps[b][1][:], lhsT=fc1rT[b][:, kk, :],
                                     rhs=w2_sb[:, 512:1024], start=(kk == 0), stop=False)
            fc2r = [smallp.tile([128, 1024], fp16, tag=f"fc2r{b}", name=f"fc2r{b}") for b in range(n_batches)]
            for b in range(n_batches):
                nc.tensor.matmul(out=fc2_ps[b][0][:], lhsT=ones, rhs=bias_sb[:, 1024:1536],
                                 start=False, stop=True)
                nc.tensor.matmul(out=fc2_ps[b][1][:], lhsT=ones, rhs=bias_sb[:, 1536:2048],
                                 start=False, stop=True)
                nc.scalar.activation(out=fc2r[b][:, 0:512], in_=fc2_ps[b][0][:], func=RELU)
                nc.scalar.activation(out=fc2r[b][:, 512:1024], in_=fc2_ps[b][1][:], func=RELU)

            fc2rT = [smallp.tile([128, 8, 128], fp16, tag=f"fc2rT{b}", name=f"fc2rT{b}") for b in range(n_batches)]
            for b in range(n_batches):
                for kk in range(8):
                    tp = psmm.tile([128, 128], fp16, tag="mm", name=f"tp2_{b}_{kk}")
                    nc.tensor.transpose(out=tp[:], in_=fc2r[b][:, kk * 128:(kk + 1) * 128],
                                        identity=ident[:])
                    if kk % 2 == 0:
                        nc.vector.tensor_copy(out=fc2rT[b][:, kk, :], in_=tp[:])
                    else:
                        nc.scalar.copy(out=fc2rT[b][:, kk, :], in_=tp[:])

            # ---------------- fc3 ----------------
            w3_sb = persist.tile([128, 8, 32], fp16, tag="w3s")
            nc.sync.dma_start(out=w3_sb[:], in_=w3[:].rearrange("p (k n) -> p k n", k=8))
            for b in range(n_batches):
                fc3_ps = psmm.tile([128, 32], fp32, tag="mm", name=f"fc3ps{b}")
                for kk in range(8):
                    nc.tensor.matmul(out=fc3_ps[:], lhsT=fc2rT[b][:, kk, :],
                                     rhs=w3_sb[:, kk, :], start=(kk == 0), stop=False)
                nc.tensor.matmul(out=fc3_ps[:], lhsT=ones, rhs=bias_sb[:, 2048:2080],
                                 start=False, stop=True)
                out_sb = smallp.tile([128, 20], fp32, tag=f"out_sb{b}", name=f"out_sb{b}")
                nc.vector.tensor_copy(out=out_sb[:], in_=fc3_ps[:, 0:20])
                nc.sync.dma_start(out=out_t[b * 128:(b + 1) * 128, :], in_=out_sb[:])

    nc.compile()
    return nc




def _install_ntff_shim():
    """The agent image's antenv lacks axon_hooks; recreate the NTFF profile hook
    via ctypes against the axon PJRT .so (same ABI trn_boot uses)."""
    import sys, types, ctypes, contextlib, os
    if "antenv.axon_hooks" in sys.modules:
        return
    so_path = "/opt/axon/libaxon_pjrt.so"
    if not os.path.exists(so_path):
        return
    lib = ctypes.CDLL(so_path)
    if not hasattr(lib, "axon_start_nrt_profile"):
        return
    lib.axon_start_nrt_profile.argtypes = [ctypes.POINTER(ctypes.c_int64), ctypes.c_size_t]
    lib.axon_start_nrt_profile.restype = ctypes.c_int64
    lib.axon_stop_nrt_profile.argtypes = [ctypes.c_char_p]
    lib.axon_stop_nrt_profile.restype = ctypes.c_int64

    @contextlib.contextmanager
    def _hook(output_dir, device_ids):
        import jax
        jax.devices()
        if device_ids:
            ids = (ctypes.c_int64 * len(device_ids))(*device_ids)
            rc = lib.axon_start_nrt_profile(ids, len(device_ids))
        else:
            rc = lib.axon_start_nrt_profile(None, 0)
        if rc != 0:
            raise RuntimeError(f"axon_start_nrt_profile rc={rc}")
        try:
            yield
        finally:
            n = lib.axon_stop_nrt_profile(str(output_dir).encode())
            print(f"ntff profile: {n} file(s) -> {output_dir}", file=sys.stderr)

    mod = types.ModuleType("antenv.axon_hooks")
    mod.get_axon_ntff_profile_hook = lambda: _hook
    mod.set_axon_ntff_profile_hook = lambda h: None
    sys.modules["antenv.axon_hooks"] = mod


# ---------------------------------------------------------------- entry point
def kernel(fm0, fm1, fm2, fm3, rois, fc1_w, fc1_b, fc2_w, fc2_b, p_w, p_b):
    from concourse.bass_utils import run_bass_kernel_spmd

    fms = [np.asarray(f, dtype=np.float32) for f in (fm0, fm1, fm2, fm3)]
    rois = np.asarray(rois, dtype=np.float32)
    n = rois.shape[0]

    fm0_nhwc = fms[0].transpose(0, 2, 3, 1)  # [B, H, W, C]
    table = np.concatenate(
        [f.transpose(0, 2, 3, 1).reshape(-1, C) for f in fms]
        + [fm0_nhwc.transpose(2, 0, 1, 3).reshape(-1, C)],  # X-major: [W, B, H, C]
        axis=0,
    ).astype(np.float16)
    assert table.shape[0] == TOTAL_ROWS_EXT

    per_roi = _prep_rois(rois)
    structure, core_slots = _pack(per_roi)
    total_slots = sum(len(tc) for _, tc in structure)
    n_batches = (total_slots + 127) // 128
    assert n_batches <= 2, f"too many slots: {total_slots}"

    w1p = np.ascontiguousarray(
        np.asarray(fc1_w, np.float32).reshape(1024, 2, 128, NBINS)
        .transpose(1, 3, 2, 0).reshape(98 * 128, 1024)
    ).astype(np.float16)
    w2p = np.ascontiguousarray(np.asarray(fc2_w, np.float32).T).astype(np.float16)
    w3_full = np.zeros((8 * 128, 32), dtype=np.float16)
    w3_full[:, :20] = np.asarray(p_w, np.float32).T.astype(np.float16)
    w3p = np.ascontiguousarray(
        w3_full.reshape(8, 128, 32).transpose(1, 0, 2).reshape(128, 8 * 32))
    biases = np.zeros((1, 2 * 1024 + 32 + 128), dtype=np.float16)
    biases[0, 0:1024] = np.asarray(fc1_b, np.float32).astype(np.float16)
    biases[0, 1024:2048] = np.asarray(fc2_b, np.float32).astype(np.float16)
    biases[0, 2048:2068] = np.asarray(p_b, np.float32).astype(np.float16)
    biases[0, 2080:2208] = np.float16(1.0)

    in_maps = []
    core_roi_ids = []
    idx_shapes = None
    n_wtiles = None
    for cc in range(N_CORES):
        idx_arrays, W_sw, roi_ids = _build_core_data(structure, core_slots[cc])
        if idx_shapes is None:
            idx_shapes = [a.shape[1] for a in idx_arrays]
            n_wtiles = W_sw.shape[1] // NBINS
        m = {"table": table, "w_pool": W_sw, "w1": w1p, "w2": w2p, "w3": w3p,
             "biases": biases}
        for we, a in enumerate(idx_arrays):
            m[f"idx{we}"] = a if a.shape[1] else np.zeros((128, 1), np.int16)
        in_maps.append(m)
        core_roi_ids.append(roi_ids)

    key = tuple((wi, tuple(tc)) for wi, tc in structure)
    if key not in _BUILD_CACHE:
        _BUILD_CACHE.clear()
        _BUILD_CACHE[key] = _build_program(structure, idx_shapes, n_wtiles, n_batches)
    nc = _BUILD_CACHE[key]

    import os
    trace_env = os.environ.get("KERNEL_TRACE", "0") == "1"
    if trace_env:
        _install_ntff_shim()
    res = run_bass_kernel_spmd(nc, in_maps, core_ids=list(range(N_CORES)),
                               trace=trace_env,
                               trace_cores=list(range(N_CORES)) if trace_env else None)
    global LAST_RESULT
    LAST_RESULT = res

    out = np.zeros((n, 20), dtype=np.float32)
    for cc in range(N_CORES):
        rows = res.results[cc]["out"]
        for s, rid in enumerate(core_roi_ids[cc]):
            if rid >= 0:
                out[rid] = rows[s]
    return out


# revision 18
# speedup vs baseline: 1.2670x; 1.2670x over previous
"""Trainium2 Bass kernel for Cascade R-CNN box head (ROIAlign over FPN + 3-layer MLP).

Sharding: data-parallel over ROIs across 8 NeuronCores, stratified by pixel-table
window so all cores run one SPMD program; FC weights replicated (fp16).

Per core the device:
  1. dma_gather's each ROI's unique bilinear-corner pixel vectors [pix, C=256] (fp16)
     from an NHWC pixel table.
  2. Pools via PE matmuls: lhsT = pixel tile (stationary), rhs = host-built separable
     ROIAlign weights [pix, 49] -> PSUM [c_half, 49], accumulated over the ROI's tiles.
  3. Assembles pooled^T in [c', (roi, k)] layout and runs fp16 GEMMs:
     12544->1024 +relu, 1024->1024 +relu, 1024->20, with PE transposes between.
"""
import numpy as np

OUT = 7
RATIO = 2
P = OUT * RATIO  # 14 sample points per dim
STRIDES = (4, 8, 16, 32)
SHAPES = {0: (192, 320), 1: (96, 160), 2: (48, 80), 3: (24, 40)}
B = 2
C = 256
NBINS = OUT * OUT  # 49
N_CORES = 8
MAX_CHUNK_TILES = 12  # pixel tiles per dma_gather chunk (12*128*512B = 768KB)

_BUILD_CACHE = {}
LAST_RESULT = None

# ---------------------------------------------------------------- windows
_LVL_BASE = {}
_base = 0
for _l in range(4):
    _H, _W = SHAPES[_l]
    _LVL_BASE[_l] = _base
    _base += B * _H * _W
TOTAL_ROWS = _base  # 163200 (Y-major: row = lvl_base + b*H*W + y*W + x)
# L0 is additionally stored X-major (row = X_BASE + x*(B*H) + b*H + y) for
# tall-skinny ROIs: at lvl0 span_x*span_y < 784 so at most one dim is long.
X_BASE = TOTAL_ROWS
TOTAL_ROWS_EXT = TOTAL_ROWS + B * 192 * 320  # 286080

# (start, nrows). Y-major L0 windows: ROI fits iff its row span <= 31 (idx span
# <= 30*320+319 = 9919); stride 32768-9920. X-major: x-extent <= 30 -> idx span
# <= 29*384+191 < 11328; stride 32768-11520.
WINDOWS = []
_l0_end = _LVL_BASE[1]
_s = 0
while True:
    if _s + 32768 >= _l0_end:
        WINDOWS.append((_l0_end - 32768, 32768))
        break
    WINDOWS.append((_s, 32768))
    _s += 32768 - 9920
_N_YWIN = len(WINDOWS)
_s = 0
while True:
    if _s + 32768 >= B * 192 * 320:
        WINDOWS.append((X_BASE + B * 192 * 320 - 32768, 32768))
        break
    WINDOWS.append((X_BASE + _s, 32768))
    _s += 32768 - 11520
WINDOWS.append((_LVL_BASE[1], B * 96 * 160))          # L1: 30720 rows
WINDOWS.append((_LVL_BASE[2], B * (48 * 80 + 24 * 40)))  # L2+L3: 9600 rows


def _find_window(lo, hi):
    for wi, (start, nrows) in enumerate(WINDOWS):
        if lo >= start and hi < start + nrows:
            return wi
    return -1


# ---------------------------------------------------------------- host-side math
def _axis_r(coord, size):
    """Mirror of reference._bilinear_axis, collapsed to unique indices.
    Returns unique indices U and R [len(U), 7] with the 0.5-bin-average folded in."""
    coord = coord.astype(np.float32)
    valid = ((coord > -1.0) & (coord < size)).astype(np.float32)
    c = np.clip(coord, np.float32(0.0), np.float32(size - 1))
    i0 = np.floor(c).astype(np.int32)
    i1 = np.minimum(i0 + 1, size - 1)
    frac = (c - i0.astype(np.float32)).astype(np.float32)
    w0 = ((np.float32(1.0) - frac) * valid).astype(np.float32)
    w1 = (frac * valid).astype(np.float32)
    uniq = np.unique(np.concatenate([i0, i1]))
    pos = {int(v): k for k, v in enumerate(uniq)}
    R = np.zeros((len(uniq), OUT), dtype=np.float32)
    for s in range(P):
        o = s // RATIO
        R[pos[int(i0[s])], o] += np.float32(0.5) * w0[s]
        R[pos[int(i1[s])], o] += np.float32(0.5) * w1[s]
    return uniq, R


def _prep_rois(rois):
    rois = np.asarray(rois, dtype=np.float32)
    n = rois.shape[0]
    bidx = rois[:, 0].astype(np.int32)
    boxes = rois[:, 1:5].astype(np.float32)
    w = boxes[:, 2] - boxes[:, 0]
    h = boxes[:, 3] - boxes[:, 1]
    k = np.floor(np.float32(4.0) + np.log2(np.sqrt(np.maximum(w * h, np.float32(1e-6))) / np.float32(224.0)))
    lvl = np.clip(k, 2.0, 5.0).astype(np.int32) - 2
    steps = ((np.arange(P, dtype=np.float32) + np.float32(0.5)) / np.float32(P)).astype(np.float32)
    per_roi = []
    for i in range(n):
        l = int(lvl[i])
        stride = np.float32(STRIDES[l])
        H, W = SHAPES[l]
        x1 = boxes[i, 0] / stride - np.float32(0.5)
        y1 = boxes[i, 1] / stride - np.float32(0.5)
        x2 = boxes[i, 2] / stride - np.float32(0.5)
        y2 = boxes[i, 3] / stride - np.float32(0.5)
        px = (x1 + (x2 - x1) * steps).astype(np.float32)
        py = (y1 + (y2 - y1) * steps).astype(np.float32)
        ux, RX = _axis_r(px, W)
        uy, RY = _axis_r(py, H)
        base = _LVL_BASE[l] + int(bidx[i]) * H * W
        gidx = (base + uy[:, None].astype(np.int64) * W + ux[None, :]).ravel()
        wi = _find_window(int(gidx.min()), int(gidx.max()))
        if wi >= 0:
            Wd = np.einsum("yo,xp->yxop", RY, RX).reshape(-1, NBINS)
        else:
            # X-major fallback (lvl0 tall-skinny): row = X_BASE + x*384 + b*192 + y
            assert l == 0, (l, int(gidx.min()), int(gidx.max()))
            gidx = (X_BASE + ux[:, None].astype(np.int64) * (B * H)
                    + int(bidx[i]) * H + uy[None, :]).ravel()
            wi = _find_window(int(gidx.min()), int(gidx.max()))
            assert wi >= 0, (int(gidx.min()), int(gidx.max()))
            Wd = np.einsum("xp,yo->xyop", RX, RY).reshape(-1, NBINS)
        npix = len(gidx)
        per_roi.append(dict(rid=i, win=wi, npix=npix, ntiles=(npix + 127) // 128,
                            lidx=(gidx - WINDOWS[wi][0]).astype(np.int16), Wd=Wd))
    return per_roi


def _pack(per_roi):
    """structure: list of (window, [per-slot tile counts]) shared by all cores;
    core_slots: per core, per global slot, the roi dict or None (dummy)."""
    by_win = {}
    for r in per_roi:
        by_win.setdefault(r["win"], []).append(r)
    structure = []
    core_slots = [[] for _ in range(N_CORES)]
    for wi in sorted(by_win):
        rs = sorted(by_win[wi], key=lambda r: -r["ntiles"])
        nslots = (len(rs) + N_CORES - 1) // N_CORES
        tile_counts = []
        for s in range(nslots):
            group = rs[s * N_CORES:(s + 1) * N_CORES]
            tile_counts.append(group[0]["ntiles"])
            for c in range(N_CORES):
                core_slots[c].append(group[c] if c < len(group) else None)
        structure.append((wi, tile_counts))
    return structure, core_slots


def _chunk_plan(tile_counts):
    """Greedy chunks of whole slots, each <= MAX_CHUNK_TILES tiles.
    Returns list of (chunk_tile_count, idx_col_offset, idx_cols, first_slot, n_slots)."""
    plan = []
    cur_slots = 0
    cur_t = 0
    first = 0
    col = 0
    for k, t in enumerate(tile_counts):
        assert t <= MAX_CHUNK_TILES
        if cur_slots and cur_t + t > MAX_CHUNK_TILES:
            plan.append((cur_t, col, cur_t * 8, first, cur_slots))
            col += cur_t * 8
            first, cur_slots, cur_t = k, 0, 0
        cur_slots += 1
        cur_t += t
    if cur_slots:
        plan.append((cur_t, col, cur_t * 8, first, cur_slots))
    return plan


def _build_core_data(structure, slots):
    idx_arrays = []
    w_blocks = []
    roi_ids = []
    si = 0
    for wi, tile_counts in structure:
        plan = _chunk_plan(tile_counts)
        win_cols = []
        for (ct, col_off, cols, first, ns) in plan:
            nidx = ct * 128
            ids = np.zeros(nidx, dtype=np.int16)
            off = 0
            for k in range(first, first + ns):
                s = slots[si + k]
                if s is not None:
                    ids[off:off + s["npix"]] = s["lidx"]
                off += tile_counts[k] * 128
            wrapped = ids.reshape(nidx // 16, 16).T.copy()  # idx i -> [i%16, i//16]
            win_cols.append(wrapped)
        wrapped_all = (np.concatenate(win_cols, axis=1) if win_cols
                       else np.zeros((16, 0), np.int16))
        idx_arrays.append(np.ascontiguousarray(np.tile(wrapped_all, (8, 1))))
        for k, t in enumerate(tile_counts):
            s = slots[si + k]
            blk = np.zeros((t * 128, NBINS), dtype=np.float16)
            if s is not None:
                blk[:s["npix"]] = s["Wd"].astype(np.float16)
            w_blocks.append(blk)
            roi_ids.append(s["rid"] if s is not None else -1)
        si += len(tile_counts)
    W_all = np.concatenate(w_blocks, axis=0)              # [T*128, 49]
    T = W_all.shape[0] // 128
    W_sw = np.ascontiguousarray(
        W_all.reshape(T, 128, NBINS).transpose(1, 0, 2).reshape(128, T * NBINS)
    ).astype(np.float16)
    return idx_arrays, W_sw, roi_ids


# ---------------------------------------------------------------- device program
def _build_program(structure, idx_shapes, n_wtiles, n_batches):
    import concourse.bacc as bacc
    import concourse.mybir as mybir
    import concourse.tile as tile
    from concourse._compat import get_trn_type
    from concourse.masks import make_identity

    fp16 = mybir.dt.float16
    fp32 = mybir.dt.float32
    RELU = mybir.ActivationFunctionType.Relu

    nc = bacc.Bacc(get_trn_type() or "TRN2", num_swdge_queues=4)
    table = nc.dram_tensor("table", [TOTAL_ROWS_EXT, C], fp16, kind="ExternalInput")
    idx_ts = [nc.dram_tensor(f"idx{w}", [128, max(sh, 1)], mybir.dt.int16, kind="ExternalInput")
              for w, sh in enumerate(idx_shapes)]
    w_pool = nc.dram_tensor("w_pool", [128, n_wtiles * NBINS], fp16, kind="ExternalInput")
    w1 = nc.dram_tensor("w1", [98 * 128, 1024], fp16, kind="ExternalInput")
    w2 = nc.dram_tensor("w2", [8 * 128, 1024], fp16, kind="ExternalInput")
    w3 = nc.dram_tensor("w3", [128, 8 * 32], fp16, kind="ExternalInput")  # swizzled
    # biases: [fc1_b(1024) fc2_b(1024) p_b(32) ones(128)]
    biases = nc.dram_tensor("biases", [1, 2 * 1024 + 32 + 128], fp16, kind="ExternalInput")
    out_t = nc.dram_tensor("out", [256, 20], fp32, kind="ExternalOutput")

    with tile.TileContext(nc) as tc:
        with (
            tc.tile_pool(name="persist", bufs=1) as persist,
            tc.tile_pool(name="pix", bufs=6) as pixp,
            tc.tile_pool(name="wts", bufs=6) as wtsp,
            tc.tile_pool(name="idxp", bufs=2) as idxp,
            tc.tile_pool(name="pool_ps", bufs=4, space="PSUM") as psp,
            tc.tile_pool(name="mm_ps", bufs=4, space="PSUM") as psmm,
            tc.tile_pool(name="stream", bufs=32) as streamp,
            tc.tile_pool(name="small", bufs=1) as smallp,
        ):
            pooledT = [persist.tile([128, 128, 98], fp16, tag=f"pooledT{b}", name=f"pooledT{b}")
                       for b in range(n_batches)]
            for b in range(n_batches):
                eng = nc.vector if b == 0 else nc.gpsimd
                eng.memset(pooledT[b][:], 0.0)
            ident = persist.tile([128, 128], fp16, tag="ident")
            make_identity(nc, ident[:])
            bias_sb = persist.tile([1, 2 * 1024 + 32 + 128], fp16, tag="bias")
            nc.sync.dma_start(out=bias_sb[:], in_=biases[:])
            ones = bias_sb[:, 2080:2208]  # [1, 128] of 1.0

            # ---------------- pooling ----------------
            slot_base = 0
            wtile_base = 0
            gq = 0
            for we, (wi, tile_counts) in enumerate(structure):
                win_start, win_rows = WINDOWS[wi]
                win_ap = table[win_start:win_start + win_rows, :]
                idx_sb = idxp.tile([128, max(idx_shapes[we], 1)], mybir.dt.int16, tag="idx")
                nc.scalar.dma_start(out=idx_sb[:], in_=idx_ts[we][:])
                for (ct, col_off, cols, first, ns) in _chunk_plan(tile_counts):
                    pix = pixp.tile([128, MAX_CHUNK_TILES, C], fp16, tag="pix")
                    nc.gpsimd.dma_gather(
                        pix[:, :ct, :], win_ap, idx_sb[:, col_off:col_off + cols],
                        ct * 128, ct * 128, C, single_packet=False,
                        queue_num=gq % 4,
                    )
                    gq += 1
                    wts = wtsp.tile([128, MAX_CHUNK_TILES * NBINS], fp16, tag="wts")
                    nc.scalar.dma_start(
                        out=wts[:, :ct * NBINS],
                        in_=w_pool[:, wtile_base * NBINS:(wtile_base + ct) * NBINS],
                    )
                    t_off = 0
                    for k in range(first, first + ns):
                        t = tile_counts[k]
                        ps0 = psp.tile([128, NBINS], fp32, tag="pool_ps")
                        ps1 = psp.tile([128, NBINS], fp32, tag="pool_ps")
                        for tt in range(t):
                            rhs = wts[:, (t_off + tt) * NBINS:(t_off + tt + 1) * NBINS]
                            nc.tensor.matmul(out=ps0[:], lhsT=pix[:, t_off + tt, 0:128],
                                             rhs=rhs, start=(tt == 0), stop=(tt == t - 1))
                            nc.tensor.matmul(out=ps1[:], lhsT=pix[:, t_off + tt, 128:256],
                                             rhs=rhs, start=(tt == 0), stop=(tt == t - 1))
                        r = slot_base + k
                        dst = pooledT[r // 128]
                        nc.vector.tensor_copy(out=dst[:, r % 128, 0:49], in_=ps0[:])
                        nc.scalar.copy(out=dst[:, r % 128, 49:98], in_=ps1[:])
                        t_off += t
                    wtile_base += ct
                slot_base += len(tile_counts)

            # ---------------- fc1 ----------------
            fc1_ps = [[psmm.tile([128, 512], fp32, tag="mm", name=f"fc1ps{b}_{h}") for h in range(2)]
                      for b in range(n_batches)]
            for kk in range(98):
                w1_sb = streamp.tile([128, 1024], fp16, tag="wstream")
                nc.sync.dma_start(out=w1_sb[:], in_=w1[kk * 128:(kk + 1) * 128, :])
                for b in range(n_batches):
                    nc.tensor.matmul(out=fc1_ps[b][0][:], lhsT=pooledT[b][:, :, kk],
                                     rhs=w1_sb[:, 0:512], start=(kk == 0), stop=False)
                    nc.tensor.matmul(out=fc1_ps[b][1][:], lhsT=pooledT[b][:, :, kk],
                                     rhs=w1_sb[:, 512:1024], start=(kk == 0), stop=False)
            fc1r = [smallp.tile([128, 1024], fp16, tag=f"fc1r{b}", name=f"fc1r{b}") for b in range(n_batches)]
            for b in range(n_batches):
                nc.tensor.matmul(out=fc1_ps[b][0][:], lhsT=ones, rhs=bias_sb[:, 0:512],
                                 start=False, stop=True)
                nc.tensor.matmul(out=fc1_ps[b][1][:], lhsT=ones, rhs=bias_sb[:, 512:1024],
                                 start=False, stop=True)
                nc.scalar.activation(out=fc1r[b][:, 0:512], in_=fc1_ps[b][0][:], func=RELU)
                nc.scalar.activation(out=fc1r[b][:, 512:1024], in_=fc1_ps[b][1][:], func=RELU)

            fc1rT = [smallp.tile([128, 8, 128], fp16, tag=f"fc1rT{b}", name=f"fc1rT{b}") for b in range(n_batches)]
            for b in range(n_batches):
                for kk in range(8):
                    tp = psmm.tile([128, 128], fp16, tag="mm", name=f"tp1_{b}_{kk}")
                    nc.tensor.transpose(out=tp[:], in_=fc1r[b][:, kk * 128:(kk + 1) * 128],
                                        identity=ident[:])
                    if kk % 2 == 0:
                        nc.vector.tensor_copy(out=fc1rT[b][:, kk, :], in_=tp[:])
                    else:
                        nc.scalar.copy(out=fc1rT[b][:, kk, :], in_=tp[:])

            # ---------------- fc2 ----------------
            fc2_ps = [[psmm.tile([128, 512], fp32, tag="mm", name=f"fc2ps{b}_{h}") for h in range(2)]
                      for b in range(n_batches)]
            for kk in range(8):
                w2_sb = streamp.tile([128, 1024], fp16, tag="wstream")
                nc.sync.dma_start(out=w2_sb[:], in_=w2[kk * 128:(kk + 1) * 128, :])
                for b in range(n_batches):
                    nc.tensor.matmul(out=fc2_ps[b][0][:], lhsT=fc1rT[b][:, kk, :],
                                     rhs=w2_sb[:, 0:512], start=(kk == 0), stop=False)
                    nc.tensor.matmul(out=fc2_ps[b][1][:], lhsT=fc1rT[b][:, kk, :],
                                     rhs=w2_sb[:, 512:1024], start=(kk == 0), stop=False)
            fc2r = [smallp.tile([128, 1024], fp16, tag=f"fc2r{b}", name=f"fc2r{b}") for b in range(n_batches)]
            for b in range(n_batches):
                nc.tensor.matmul(out=fc2_ps[b][0][:], lhsT=ones, rhs=bias_sb[:, 1024:1536],
                                 start=False, stop=True)
                nc.tensor.matmul(out=fc2_ps[b][1][:], lhsT=ones, rhs=bias_sb[:, 1536:2048],
                                 start=False, stop=True)
                nc.scalar.activation(out=fc2r[b][:, 0:512], in_=fc2_ps[b][0][:], func=RELU)
                nc.scalar.activation(out=fc2r[b][:, 512:1024], in_=fc2_ps[b][1][:], func=RELU)

            fc2rT = [smallp.tile([128, 8, 128], fp16, tag=f"fc2rT{b}", name=f"fc2rT{b}") for b in range(n_batches)]
            for b in range(n_batches):
                for kk in range(8):
                    tp = psmm.tile([128, 128], fp16, tag="mm", name=f"tp2_{b}_{kk}")
                    nc.tensor.transpose(out=tp[:], in_=fc2r[b][:, kk * 128:(kk + 1) * 128],
                                        identity=ident[:])
                    if kk % 2 == 0:
                        nc.vector.tensor_copy(out=fc2rT[b][:, kk, :], in_=tp[:])
                    else:
                        nc.scalar.copy(out=fc2rT[b][:, kk, :], in_=tp[:])

            # ---------------- fc3 ----------------
            w3_sb = persist.tile([128, 8, 32], fp16, tag="w3s")
            nc.sync.dma_start(out=w3_sb[:], in_=w3[:].rearrange("p (k n) -> p k n", k=8))
            for b in range(n_batches):
                fc3_ps = psmm.tile([128, 32], fp32, tag="mm", name=f"fc3ps{b}")
                for kk in range(8):
                    nc.tensor.matmul(out=fc3_ps[:], lhsT=fc2rT[b][:, kk, :],
                                     rhs=w3_sb[:, kk, :], start=(kk == 0), stop=False)
                nc.tensor.matmul(out=fc3_ps[:], lhsT=ones, rhs=bias_sb[:, 2048:2080],
                                 start=False, stop=True)
                out_sb = smallp.tile([128, 20], fp32, tag=f"out_sb{b}", name=f"out_sb{b}")
                nc.vector.tensor_copy(out=out_sb[:], in_=fc3_ps[:, 0:20])
                nc.sync.dma_start(out=out_t[b * 128:(b + 1) * 128, :], in_=out_sb[:])

    nc.compile()
    return nc




def _install_ntff_shim():
    """The agent image's antenv lacks axon_hooks; recreate the NTFF profile hook
    via ctypes against the axon PJRT .so (same ABI trn_boot uses)."""
    import sys, types, ctypes, contextlib, os
    if "antenv.axon_hooks" in sys.modules:
        return
    so_path = "/opt/axon/libaxon_pjrt.so"
    if not os.path.exists(so_path):
        return
    lib = ctypes.CDLL(so_path)
    if not hasattr(lib, "axon_start_nrt_profile"):
        return
    lib.axon_start_nrt_profile.argtypes = [ctypes.POINTER(ctypes.c_int64), ctypes.c_size_t]
    lib.axon_start_nrt_profile.restype = ctypes.c_int64
    lib.axon_stop_nrt_profile.argtypes = [ctypes.c_char_p]
    lib.axon_stop_nrt_profile.restype = ctypes.c_int64

    @contextlib.contextmanager
    def _hook(output_dir, device_ids):
        import jax
        jax.devices()
        if device_ids:
            ids = (ctypes.c_int64 * len(device_ids))(*device_ids)
            rc = lib.axon_start_nrt_profile(ids, len(device_ids))
        else:
            rc = lib.axon_start_nrt_profile(None, 0)
        if rc != 0:
            raise RuntimeError(f"axon_start_nrt_profile rc={rc}")
        try:
            yield
        finally:
            n = lib.axon_stop_nrt_profile(str(output_dir).encode())
            print(f"ntff profile: {n} file(s) -> {output_dir}", file=sys.stderr)

    mod = types.ModuleType("antenv.axon_hooks")
    mod.get_axon_ntff_profile_hook = lambda: _hook
    mod.set_axon_ntff_profile_hook = lambda h: None
    sys.modules["antenv.axon_hooks"] = mod


# ---------------------------------------------------------------- entry point
def kernel(fm0, fm1, fm2, fm3, rois, fc1_w, fc1_b, fc2_w, fc2_b, p_w, p_b):
    from concourse.bass_utils import run_bass_kernel_spmd

    fms = [np.asarray(f, dtype=np.float32) for f in (fm0, fm1, fm2, fm3)]
    rois = np.asarray(rois, dtype=np.float32)
    n = rois.shape[0]

    fm0_nhwc = fms[0].transpose(0, 2, 3, 1)  # [B, H, W, C]
    table = np.concatenate(
        [f.transpose(0, 2, 3, 1).reshape(-1, C) for f in fms]
        + [fm0_nhwc.transpose(2, 0, 1, 3).reshape(-1, C)],  # X-major: [W, B, H, C]
        axis=0,
    ).astype(np.float16)
    assert table.shape[0] == TOTAL_ROWS_EXT

    per_roi = _prep_rois(rois)
    structure, core_slots = _pack(per_roi)
    total_slots = sum(len(tc) for _, tc in structure)
    n_batches = (total_slots + 127) // 128
    assert n_batches <= 2, f"too many slots: {total_slots}"

    w1p = np.ascontiguousarray(
        np.asarray(fc1_w, np.float32).reshape(1024, 2, 128, NBINS)
        .transpose(1, 3, 2, 0).reshape(98 * 128, 1024)
    ).astype(np.float16)
    w2p = np.ascontiguousarray(np.asarray(fc2_w, np.float32).T).astype(np.float16)
    w3_full = np.zeros((8 * 128, 32), dtype=np.float16)
    w3_full[:, :20] = np.asarray(p_w, np.float32).T.astype(np.float16)
    w3p = np.ascontiguousarray(
        w3_full.reshape(8, 128, 32).transpose(1, 0, 2).reshape(128, 8 * 32))
    biases = np.zeros((1, 2 * 1024 + 32 + 128), dtype=np.float16)
    biases[0, 0:1024] = np.asarray(fc1_b, np.float32).astype(np.float16)
    biases[0, 1024:2048] = np.asarray(fc2_b, np.float32).astype(np.float16)
    biases[0, 2048:2068] = np.asarray(p_b, np.float32).astype(np.float16)
    biases[0, 2080:2208] = np.float16(1.0)

    in_maps = []
    core_roi_ids = []
    idx_shapes = None
    n_wtiles = None
    for cc in range(N_CORES):
        idx_arrays, W_sw, roi_ids = _build_core_data(structure, core_slots[cc])
        if idx_shapes is None:
            idx_shapes = [a.shape[1] for a in idx_arrays]
            n_wtiles = W_sw.shape[1] // NBINS
        m = {"table": table, "w_pool": W_sw, "w1": w1p, "w2": w2p, "w3": w3p,
             "biases": biases}
        for we, a in enumerate(idx_arrays):
            m[f"idx{we}"] = a if a.shape[1] else np.zeros((128, 1), np.int16)
        in_maps.append(m)
        core_roi_ids.append(roi_ids)

    key = tuple((wi, tuple(tc)) for wi, tc in structure)
    if key not in _BUILD_CACHE:
        _BUILD_CACHE.clear()
        _BUILD_CACHE[key] = _build_program(structure, idx_shapes, n_wtiles, n_batches)
    nc = _BUILD_CACHE[key]

    import os
    trace_env = os.environ.get("KERNEL_TRACE", "0") == "1"
    if trace_env:
        _install_ntff_shim()
    res = run_bass_kernel_spmd(nc, in_maps, core_ids=list(range(N_CORES)),
                               trace=trace_env,
                               trace_cores=list(range(N_CORES)) if trace_env else None)
    global LAST_RESULT
    LAST_RESULT = res

    out = np.zeros((n, 20), dtype=np.float32)
    for cc in range(N_CORES):
        rows = res.results[cc]["out"]
        for s, rid in enumerate(core_roi_ids[cc]):
            if rid >= 0:
                out[rid] = rows[s]
    return out
